# revision 17
# baseline (speedup 1.0000x reference)
"""Trainium2 Bass kernel for BasicBiPointNetPartSeg (8 NeuronCores).

Sharding: pure data parallel over batch B=8 (one batch element per core),
activations laid out [C, N] (channels on partitions, 2048 points on free dim).

Key points
----------
- Binary convs are exact integer GEMMs: signs in bf16/fp8, fp32 PSUM.
- BN batch-stats: per-core (sum [, sumsq]) partials AllGather'd across the 8
  cores and reduced locally (AG has the lowest collective floor on-chip).
- Faithful fp32 rounding vs the JAX reference: conv bias is added to the
  integer conv output with a single fp32 rounding (like the reference),
  stats are computed over those biased values, and every sign-only BN layer
  uses sign(y - mean) (beta == 0 in this model, so variance is irrelevant).
  Variance (with one Newton-refined sqrt) is only computed for the two
  layers whose BN *values* are consumed (main bn3 -> net_t, seg bns3 -> s4).
- max-pools commute with the monotone BN affine (gamma > 0): pool raw
  integers locally, fix up bias/mean on the pooled [C] vector.
- The two STN fc stacks run replicated over the batch from the AllGather'd
  pooled sign vectors (their BN-over-batch becomes core-local); each core
  extracts its own transform via its partition id through a DRAM bounce.
- s1's contribution from the broadcast pooled/label block is a per-(batch,
  channel) integer folded into s1's stat/sign passes.
"""

import numpy as np

import concourse.bass as bass
import concourse.bacc as bacc
import concourse.tile as tile
import concourse.mybir as mybir
import concourse.bass_utils as bass_utils
from concourse.bass import ds

dt = mybir.dt
AF = mybir.ActivationFunctionType
ALU = mybir.AluOpType
AX = mybir.AxisListType

N_CORES = 8
NPTS = 2048
EPS = 1e-5
RG = [list(range(N_CORES))]

f32, f16, bf16, f8 = dt.float32, dt.float16, dt.bfloat16, dt.float8e4


def _np(d):
    return mybir.dt.np(d)


# ===========================================================================
# device program builder
# ===========================================================================

class Prog:
    def __init__(self, npts=NPTS, debug=False):
        self.debug_mode = debug
        self.dbg_names = []
        self.npts = npts
        self.ntot = N_CORES * npts
        self.nsub = min(512, npts)
        self.ntiles = npts // self.nsub
        self.nc = bacc.Bacc("TRN2", target_bir_lowering=False, debug=False,
                            num_devices=N_CORES)
        self.inputs = {}

    def dbg(self, name, ap, shape=None):
        if not self.debug_mode:
            return
        shape = list(shape if shape is not None else ap.shape)
        t = self.nc.dram_tensor(f"dbg_{name}", shape, ap.dtype,
                                kind="ExternalOutput")
        self.nc.sync.dma_start(t.ap(), ap)
        self.dbg_names.append(f"dbg_{name}")

    def dram_in(self, name, shape, d=f32):
        t = self.nc.dram_tensor(name, list(shape), d, kind="ExternalInput")
        self.inputs[name] = (tuple(shape), _np(d))
        return t

    # ---- loading ---------------------------------------------------------
    def load_wT(self, name, c_in, c_out, d=bf16):
        """wT [C_in, C_out] dram -> [(tile [kk, C_out], kk)] per K-chunk."""
        nc = self.nc
        w = self.dram_in(name, [c_in, c_out], d)
        tiles = []
        for k in range(0, c_in, 128):
            kk = min(128, c_in - k)
            t = self.wpool.tile([kk, c_out], d, tag=f"w_{name}_{k}", name=f"w_{name}_{k}")
            nc.sync.dma_start(t[:], w.ap()[k:k + kk, :])
            tiles.append((t, kk))
        return tiles

    def load_vec(self, name, c, d=f32):
        """[nch*128] dram vector -> SBUF [128, nch]; channel c at
        [c % 128, c // 128]."""
        nc = self.nc
        nch = (c + 127) // 128
        v = self.dram_in(name, [nch * 128], d)
        t = self.spool.tile([128, nch], d, tag=f"v_{name}", name=f"v_{name}")
        nc.sync.dma_start(t[:], v.ap().rearrange("(k p) -> p k", p=128))
        return t

    # ---- collectives -----------------------------------------------------
    def allgather_stats(self, name, blocks, nch):
        nc = self.nc
        nblk = len(blocks)
        cin = self.dpool.tile([nblk, 128, nch], f32, tag=f"agi_{name}", name=f"agi_{name}")
        for i, b in enumerate(blocks):
            nc.sync.dma_start(cin[i], b[:])
        cout = self.dpool.tile([N_CORES, nblk, 128, nch], f32, tag=f"ago_{name}", name=f"ago_{name}")
        nc.gpsimd.collective_compute("AllGather", ALU.bypass, replica_groups=RG,
                                     ins=[cin.opt()], outs=[cout.opt()])
        gath = self.spool.tile([128, nblk, nch, 8], f32, tag=f"gath_{name}", name=f"gath_{name}")
        for i in range(nblk):
            nc.sync.dma_start(gath[:, i],
                              cout[:, i].rearrange("r c k -> c k r"))
        return gath

    def reduce_ranks(self, gath, blk, nch, tag):
        t = self.spool.tile([128, nch], f32, tag=f"rr_{tag}", name=f"rr_{tag}")
        self.nc.vector.tensor_reduce(t[:], gath[:, blk], axis=AX.X, op=ALU.add)
        return t

    # ---- conv + BN layer -------------------------------------------------
    def conv_layer(self, name, in_chunks, wname, c_in, c_out, bias_name, *,
                   w_dtype=f8, pool=False, raw=True, raw_is_y=False,
                   value=False, gamma_name=None, beta_name=None,
                   contrib=None, preloaded_w=None, gather_pool=False,
                   raw_dtype=f16, sign_dtype=f8, sign_tag=None, groups=1,
                   want_sign=True, val_tag="valbuf"):
        """One conv+BN layer, optionally split into channel groups, each with
        its own stats AllGather + sign pass (stats are per-channel, so the
        split is exact and pipelines AG under the next group's matmuls)."""
        nc, npts, nsub, ntiles = self.nc, self.npts, self.nsub, self.ntiles
        nch = (c_out + 127) // 128
        stag = sign_tag or f"s_{name}"

        wt = preloaded_w if preloaded_w is not None else \
            self.load_wT(wname, c_in, c_out, d=w_dtype)
        b_sb = self.load_vec(bias_name, c_out)
        kc = list(zip(wt, in_chunks))

        mean_full = self.spool.tile([128, nch], f32, tag="mean", name="mean")
        maxs = self.spool.tile([128, nch], f32, tag="maxs", name="maxs") if pool else None
        junk = self.jpool.tile([128, npts], bf16, tag="junk", name="junk")

        signs = [None] * nch
        values = []
        lay = {"name": name, "nch": nch, "c_out": c_out, "mean": mean_full,
               "bias_sb": b_sb, "maxs_local": maxs, "gath": None}

        gb = (nch + groups - 1) // groups
        granges = [list(range(g * gb, min(nch, (g + 1) * gb)))
                   for g in range(groups) if g * gb < nch]
        for gi, mrange in enumerate(granges):
            gn = len(mrange)
            sums = self.spool.tile([128, gn], f32, tag="sums", name="sums")
            sumsq = self.spool.tile([128, gn], f32, tag="ssq", name="ssq") if value else None
            if c_out % 128:
                nc.vector.memset(sums[:], 0.0)
                if value:
                    nc.vector.memset(sumsq[:], 0.0)
            raws = []
            for j, m in enumerate(mrange):
                mm = min(128, c_out - m * 128)
                ps = self.ppool.tile([mm, npts], f32, tag="ps", name="ps")
                for n in range(ntiles):
                    sl = slice(n * nsub, (n + 1) * nsub)
                    for ki, ((w, kk), (xin, kin)) in enumerate(kc):
                        assert kk == kin, f"{name}: k mismatch {kk} vs {kin}"
                        nc.tensor.matmul(ps[:, sl], w[:, m * 128:m * 128 + mm],
                                         xin[:, sl],
                                         start=(ki == 0),
                                         stop=(ki == len(kc) - 1))
                bcol = b_sb[:mm, m:m + 1]
                scol = sums[:mm, j:j + 1]
                if contrib is not None:
                    # raw = int + contrib (exact); accumulate the integer sums
                    # on the same pass (fp16 cast is exact for these ints).
                    # The biased sum is reconstructed as S_int + ntot*b after
                    # the rank reduce (the DVE accumulator ignores op1).
                    rw = self.rpool.tile([mm, npts], raw_dtype,
                                         tag=f"raw{j % 8}", name=f"raw{j}")
                    nc.vector.tensor_scalar(rw[:], ps[:], contrib[m], 0.0,
                                            op0=ALU.add, op1=ALU.add,
                                            accum_out=scol)
                    raws.append(rw)
                elif raw_is_y:
                    rw = self.rpool.tile([mm, npts], f32, tag="rawy",
                                         name="rawy")
                    nc.scalar.activation(rw[:], ps[:], AF.Identity, bias=bcol,
                                         scale=1.0, accum_out=scol)
                    raws.append(rw)
                else:
                    nc.scalar.activation(junk[:mm, :], ps[:], AF.Identity,
                                         bias=bcol, scale=1.0, accum_out=scol)
                    if raw:
                        rw = self.rpool.tile([mm, npts], raw_dtype,
                                             tag=f"raw{j % 8}", name=f"raw{j}")
                        nc.vector.tensor_scalar(rw[:], ps[:], 1.0, None,
                                                op0=ALU.mult)
                        raws.append(rw)
                if value:
                    nc.scalar.activation(junk[:mm, :], ps[:], AF.Square,
                                         bias=bcol, scale=1.0,
                                         accum_out=sumsq[:mm, j:j + 1])
                if pool:
                    nc.vector.tensor_reduce(maxs[:mm, m:m + 1], ps[:],
                                            axis=AX.X, op=ALU.max)

            blocks = [sums]
            if value:
                blocks.append(sumsq)
            if gather_pool:
                blocks.append(maxs)
            gath = self.allgather_stats(f"{name}_{gi}", blocks, gn)
            if gather_pool:
                lay["gath"] = gath
            sums_t = self.reduce_ranks(gath, 0, gn, "s")
            if contrib is not None:
                # sums were integer-only: add ntot * bias analytically
                bsl = b_sb[:, mrange[0]:mrange[0] + gn]
                nc.vector.scalar_tensor_tensor(sums_t[:], bsl,
                                               float(self.ntot), sums_t[:],
                                               op0=ALU.mult, op1=ALU.add)
            mslice = mean_full[:, mrange[0]:mrange[0] + gn]
            nc.vector.tensor_scalar(mslice, sums_t[:], 1.0 / self.ntot, None,
                                    op0=ALU.mult)
            negm = self.spool.tile([128, gn], f32, tag="negm", name="negm")
            nc.vector.tensor_scalar(negm[:], sums_t[:], -1.0 / self.ntot,
                                    None, op0=ALU.mult)
            self.dbg(f"{name}_sums{gi}", sums_t[:])

            if value:
                sumsq_t = self.reduce_ranks(gath, 1, gn, "q")
                g_sb = self.load_vec(gamma_name, c_out)
                be_sb = self.load_vec(beta_name, c_out)
                sp = self.spool
                e2 = sp.tile([128, gn], f32, tag="e2", name="e2")
                nc.vector.tensor_scalar(e2[:], sumsq_t[:], 1.0 / self.ntot,
                                        None, op0=ALU.mult)
                m2 = sp.tile([128, gn], f32, tag="m2", name="m2")
                nc.vector.tensor_tensor(m2[:], mslice, mslice, op=ALU.mult)
                var = sp.tile([128, gn], f32, tag="var", name="var")
                nc.vector.tensor_tensor(var[:], e2[:], m2[:], op=ALU.subtract)
                ve = sp.tile([128, gn], f32, tag="ve", name="ve")
                nc.vector.tensor_scalar(ve[:], var[:], EPS, None, op0=ALU.add)
                std = sp.tile([128, gn], f32, tag="std", name="std")
                nc.scalar.activation(std[:], ve[:], AF.Sqrt, bias=0.0,
                                     scale=1.0)
                rstd = sp.tile([128, gn], f32, tag="rstd", name="rstd")
                nc.vector.reciprocal(rstd[:], std[:])
                t1 = sp.tile([128, gn], f32, tag="t1", name="t1")
                nc.vector.tensor_tensor(t1[:], ve[:], rstd[:], op=ALU.mult)
                std1 = sp.tile([128, gn], f32, tag="std1", name="std1")
                nc.vector.tensor_tensor(std1[:], std[:], t1[:], op=ALU.add)
                nc.vector.tensor_scalar(std1[:], std1[:], 0.5, None,
                                        op0=ALU.mult)
                inv = sp.tile([128, gn], f32, tag="inv", name="inv")
                nc.vector.reciprocal(inv[:], std1[:])
                scale = sp.tile([128, gn], f32, tag="scaleT", name="scaleT")
                nc.vector.tensor_tensor(scale[:], g_sb[:], inv[:], op=ALU.mult)
                msc = sp.tile([128, gn], f32, tag="msc", name="msc")
                nc.vector.tensor_tensor(msc[:], mslice, scale[:], op=ALU.mult)
                bias2 = sp.tile([128, gn], f32, tag="bias2", name="bias2")
                nc.vector.tensor_tensor(bias2[:], be_sb[:], msc[:],
                                        op=ALU.subtract)

            # ---- per-group sign / value apply --------------------------
            if want_sign:
                for j, m in enumerate(mrange):
                    mm = min(128, c_out - m * 128)
                    rw = raws[j]
                    if raw_is_y:
                        y = rw[:]
                    else:
                        yb = self.ypool.tile([mm, npts], f32, tag="ybuf",
                                             name="ybuf")
                        nc.vector.tensor_scalar(yb[:], rw[:],
                                                b_sb[:mm, m:m + 1], None,
                                                op0=ALU.add)
                        y = yb[:]
                    st = self.apool.tile([mm, npts], sign_dtype,
                                         tag=f"{stag}_{m}", name=f"{stag}_{m}")
                    nc.scalar.activation(st[:], y, AF.Sign,
                                         bias=negm[:mm, j:j + 1], scale=1.0)
                    signs[m] = (st, mm)
                    self.dbg(f"{name}_s{m}", st[:])
                    if value:
                        vt = self.apool.tile([mm, npts], f32,
                                             tag=f"{val_tag}_{m}",
                                             name=f"{val_tag}_{m}")
                        nc.scalar.activation(vt[:], y, AF.Identity,
                                             bias=bias2[:mm, j:j + 1],
                                             scale=scale[:mm, j:j + 1])
                        nc.vector.tensor_scalar(vt[:], vt[:], 1.0, -1.0,
                                                op0=ALU.min, op1=ALU.max)
                        values.append((vt, mm))
                        self.dbg(f"{name}_v{m}", vt[:])

        lay["signs"] = signs
        lay["values"] = values
        return lay

    def pooled_sign(self, pref, lay, gblk):
        """Gathered STN pool: sign(fl(maxint + b) - mean) for all 8 batches
        -> [128, nch, 8] bf16."""
        nc = self.nc
        nch = lay["nch"]
        maxg = lay["gath"][:, gblk]                       # [128, nch, 8]
        b_b = lay["bias_sb"][:, :nch].unsqueeze(2).broadcast_to([128, nch, 8])
        m_b = lay["mean"][:, :nch].unsqueeze(2).broadcast_to([128, nch, 8])
        yp = self.spool.tile([128, nch, 8], f32, tag=f"yp_{pref}", name=f"yp_{pref}")
        nc.vector.tensor_tensor(yp[:], maxg, b_b, op=ALU.add)
        z = self.spool.tile([128, nch, 8], f32, tag=f"zp_{pref}", name=f"zp_{pref}")
        nc.vector.tensor_tensor(z[:], yp[:], m_b, op=ALU.subtract)
        sh = self.spool.tile([128, nch, 8], bf16, tag=f"shp_{pref}", name=f"shp_{pref}")
        nc.scalar.activation(sh[:], z[:], AF.Sign)
        return sh

    def fc_bn_sign(self, tag, fy, nch, out_dtype=bf16):
        nc = self.nc
        s = self.spool.tile([128, nch], f32, tag=f"fcs_{tag}", name=f"fcs_{tag}")
        nc.vector.tensor_reduce(s[:], fy[:], axis=AX.X, op=ALU.add)
        m = self.spool.tile([128, nch], f32, tag=f"fcm_{tag}", name=f"fcm_{tag}")
        nc.vector.tensor_scalar(m[:], s[:], 1.0 / 8.0, None, op0=ALU.mult)
        z = self.spool.tile([128, nch, 8], f32, tag=f"fcz_{tag}", name=f"fcz_{tag}")
        nc.vector.tensor_tensor(z[:], fy[:],
                                m[:].unsqueeze(2).broadcast_to([128, nch, 8]),
                                op=ALU.subtract)
        sh = self.spool.tile([128, nch, 8], out_dtype, tag=f"fcsh_{tag}", name=f"fcsh_{tag}")
        nc.scalar.activation(sh[:], z[:], AF.Sign)
        return sh

    def stn_fc(self, pref, sh_pool, k):
        """Replicated fc stack from pooled signs [128, 8, 8]. Returns own
        transform: [3,3] (k=3) or [128,128] (k=128) fp32 SBUF tile."""
        nc = self.nc
        f1w = self.load_wT(f"{pref}_f1wT", 1024, 512, d=bf16)
        f1b = self.load_vec(f"{pref}_f1b", 512)
        ps = self.ppool.tile([128, 4, 8], f32, tag="ps", name="ps")
        for m in range(4):
            for ki, (w, kk) in enumerate(f1w):
                nc.tensor.matmul(ps[:, m, :], w[:, m * 128:(m + 1) * 128],
                                 sh_pool[:, ki, :],
                                 start=(ki == 0), stop=(ki == len(f1w) - 1))
        f1y = self.spool.tile([128, 4, 8], f32, tag=f"f1y_{pref}", name=f"f1y_{pref}")
        for m in range(4):
            nc.scalar.activation(f1y[:, m, :], ps[:, m, :], AF.Identity,
                                 bias=f1b[:, m:m + 1], scale=1.0)
        self.dbg(f"{pref}_f1y", f1y[:])
        sh1 = self.fc_bn_sign(f"{pref}4", f1y, 4)
        self.dbg(f"{pref}_sh1", sh1[:])

        f2w = self.load_wT(f"{pref}_f2wT", 512, 256, d=bf16)
        f2b = self.load_vec(f"{pref}_f2b", 256)
        ps2 = self.ppool.tile([128, 2, 8], f32, tag="ps", name="ps")
        for m in range(2):
            for ki, (w, kk) in enumerate(f2w):
                nc.tensor.matmul(ps2[:, m, :], w[:, m * 128:(m + 1) * 128],
                                 sh1[:, ki, :],
                                 start=(ki == 0), stop=(ki == len(f2w) - 1))
        f2y = self.spool.tile([128, 2, 8], f32, tag=f"f2y_{pref}", name=f"f2y_{pref}")
        for m in range(2):
            nc.scalar.activation(f2y[:, m, :], ps2[:, m, :], AF.Identity,
                                 bias=f2b[:, m:m + 1], scale=1.0)

        self.dbg(f"{pref}_f2y", f2y[:])
        if k == 3:
            sh2 = self.fc_bn_sign(f"{pref}5", f2y, 2)
            f3w = self.load_wT(f"{pref}_f3wT", 256, 9, d=bf16)
            f3b = self.load_vec(f"{pref}_f3b", 9)
            eye9 = self.load_vec(f"{pref}_eye", 9)
            ps3 = self.ppool.tile([9, 8], f32, tag="ps", name="ps")
            for ki, (w, kk) in enumerate(f3w):
                nc.tensor.matmul(ps3[:], w[:], sh2[:, ki, :],
                                 start=(ki == 0), stop=(ki == 1))
            t_all = self.spool.tile([9, 8], f32, tag="t3all", name="t3all")
            nc.scalar.activation(t_all[:], ps3[:], AF.Identity,
                                 bias=f3b[:9, 0:1], scale=1.0)
            nc.vector.tensor_scalar(t_all[:], t_all[:], eye9[:9, 0:1], None,
                                    op0=ALU.add)
            td = self.dpool.tile([9, 8], f32, tag="t3d", name="t3d")
            nc.sync.dma_start(td[:], t_all[:])
            t_own = self.spool.tile([3, 3], f32, tag="t3own", name="t3own")
            nc.sync.dma_start(
                t_own[:].unsqueeze(2),
                td[:].rearrange("(a b) r -> a b r", b=3)[:, :, ds(self.rank, 1)])
            return t_own
        else:
            sh2 = self.fc_bn_sign(f"{pref}5", f2y, 2, out_dtype=f8)
            self.dbg(f"{pref}_sh2", sh2[:])
            f3w = self.dram_in(f"{pref}_f3wT", [256, 16384], f8)
            b128 = self.dram_in(f"{pref}_f3b128", [128, 128], f32)
            eye128 = self.dram_in(f"{pref}_eye128", [128, 128], f32)
            t128d = self.dpool.tile([8, 16384], f32, tag="t128d", name="t128d")
            for n in range(16384 // 2048):
                ps3 = self.ppool.tile([8, 2048], f32, tag="ps", name="ps")
                for nn in range(4):
                    lo = nn * 512
                    wtile = self.wpool.tile([128, 2, 512], f8, tag="f3w_stream", name="f3w_stream", bufs=3)
                    nc.sync.dma_start(
                        wtile[:],
                        f3w.ap()[:, n * 2048 + lo:n * 2048 + lo + 512]
                        .rearrange("(ki p) c -> p ki c", p=128))
                    for ki in range(2):
                        nc.tensor.matmul(ps3[:, lo:lo + 512], sh2[:, ki, :],
                                         wtile[:, ki, :],
                                         start=(ki == 0), stop=(ki == 1))
                t3sb = self.ypool.tile([8, 2048], f32, tag="ybuf", name="t3sb")
                nc.scalar.copy(t3sb[:], ps3[:])
                nc.sync.dma_start(t128d[:, n * 2048:(n + 1) * 2048], t3sb[:])
            t_own = self.spool.tile([128, 128], f32, tag="t128own", name="t128own")
            nc.sync.dma_start(
                t_own[:],
                t128d[:].rearrange("r (c k) -> r c k", c=128)[ds(self.rank, 1)]
                .squeeze(0))
            self.dbg("t128_raw", t_own[:])
            bt = self.spool.tile([128, 128], f32, tag="b128", name="b128")
            nc.sync.dma_start(bt[:], b128.ap())
            nc.vector.tensor_tensor(t_own[:], t_own[:], bt[:], op=ALU.add)
            et = self.spool.tile([128, 128], f32, tag="eye128", name="eye128")
            nc.sync.dma_start(et[:], eye128.ap())
            nc.vector.tensor_tensor(t_own[:], t_own[:], et[:], op=ALU.add)
            return t_own

    # ------------------------------------------------------------------
    def build(self):
        nc, npts, nsub, ntiles = self.nc, self.npts, self.nsub, self.ntiles

        x_d = self.dram_in("x", [3, npts], f32)
        out_d = nc.dram_tensor("out", [50, npts], f32, kind="ExternalOutput")

        with tile.TileContext(nc) as tc:
            with (
                tc.tile_pool(name="spool", bufs=1) as spool,
                tc.tile_pool(name="apool", bufs=1) as apool,
                tc.tile_pool(name="rpool", bufs=1) as rpool,
                tc.tile_pool(name="jpool", bufs=1) as jpool,
                tc.tile_pool(name="wpool", bufs=1) as wpool,
                tc.tile_pool(name="ypool", bufs=2) as ypool,
                tc.tile_pool(name="ppool", bufs=2, space="PSUM") as ppool,
                tc.tile_pool(name="dpool", bufs=1, space="DRAM") as dpool,
            ):
                self.spool, self.apool, self.rpool, self.jpool = spool, apool, rpool, jpool
                self.wpool, self.ypool, self.ppool, self.dpool = wpool, ypool, ppool, dpool
                self.rank = nc.partition_id()

                x_sb = spool.tile([3, npts], f32, tag="x", name="x")
                nc.sync.dma_start(x_sb[:], x_d.ap())
                sx = spool.tile([3, npts], f8, tag="sx", name="sx")
                nc.scalar.activation(sx[:], x_sb[:], AF.Sign)
                self.dbg("sx", sx[:])

                # ================= STN1 (k=3) ===========================
                l = self.conv_layer("stnc1", [(sx[:], 3)], "stn_c1wT", 3, 64,
                                    "stn_c1b", sign_tag="tA")
                sh1 = l["signs"]
                l = self.conv_layer("stnc2", [(sh1[0][0][:], 64)], "stn_c2wT",
                                    64, 128, "stn_c2b", sign_tag="tB")
                sh2 = l["signs"]
                l3 = self.conv_layer("stnc3", [(sh2[0][0][:], 128)], "stn_c3wT",
                                     128, 1024, "stn_c3b", pool=True, raw=False,
                                     gather_pool=True, want_sign=False)
                shp = self.pooled_sign("stn", l3, gblk=1)
                self.dbg("stn_shp", shp[:])
                t3_own = self.stn_fc("stn", shp, 3)
                self.dbg("t3_own", t3_own[:])

                # pc = einsum(x, trans): [3, npts] fp32
                ps = ppool.tile([3, npts], f32, tag="ps", name="ps")
                for n in range(ntiles):
                    sl = slice(n * nsub, (n + 1) * nsub)
                    nc.tensor.matmul(ps[:, sl], t3_own[:], x_sb[:, sl],
                                     start=True, stop=True)
                pc = spool.tile([3, npts], f32, tag="pcb", name="pc")
                nc.scalar.copy(pc[:], ps[:])
                self.dbg("pc", pc[:])

                # ================= main convs ===========================
                l = self.conv_layer("c1", [(pc[:], 3)], "c1wT", 3, 64, "c1b",
                                    w_dtype=f32, raw_is_y=True,
                                    sign_tag="s_c1")
                out1s = l["signs"]

                l = self.conv_layer("c2", [(out1s[0][0][:], 64)], "c2wT",
                                    64, 128, "c2b", sign_tag="s_c2")
                out2s = l["signs"]

                l = self.conv_layer("c3", [(out2s[0][0][:], 128)], "c3wT",
                                    128, 128, "c3b", value=True,
                                    gamma_name="bn3g", beta_name="bn3be",
                                    sign_tag="s_c3")
                out3s, out3v = l["signs"], l["values"]

                # ================= FSTN (k=128) =========================
                l = self.conv_layer("fc1", [(out3s[0][0][:], 128)],
                                    "fstn_c1wT", 128, 64, "fstn_c1b",
                                    sign_tag="tA")
                fh1 = l["signs"]
                l = self.conv_layer("fc2", [(fh1[0][0][:], 64)], "fstn_c2wT",
                                    64, 128, "fstn_c2b", sign_tag="tB")
                fh2 = l["signs"]
                lf3 = self.conv_layer("fc3", [(fh2[0][0][:], 128)],
                                      "fstn_c3wT", 128, 1024, "fstn_c3b",
                                      pool=True, raw=False, gather_pool=True,
                                      want_sign=False)
                fshp = self.pooled_sign("fstn", lf3, gblk=1)
                self.dbg("fstn_shp", fshp[:])
                t128_own = self.stn_fc("fstn", fshp, 128)
                self.dbg("t128_own", t128_own[:])

                # net_t = t128^T @ out3v, sign only, no stats
                nts = apool.tile([128, npts], f8, tag="net_ts", name="net_ts")
                psn = ppool.tile([128, npts], f32, tag="ps", name="ps")
                for n in range(ntiles):
                    sl = slice(n * nsub, (n + 1) * nsub)
                    nc.tensor.matmul(psn[:, sl], t128_own[:],
                                     out3v[0][0][:, sl], start=True, stop=True)
                nc.scalar.activation(nts[:], psn[:], AF.Sign)
                self.dbg("nts", nts[:])

                # ================= c4, c5 ===============================
                l = self.conv_layer("c4", [(nts[:], 128)], "c4wT", 128, 512,
                                    "c4b", sign_tag="s_c4")
                out4s = l["signs"]

                l5 = self.conv_layer("c5", [(t[0][:], t[1]) for t in out4s],
                                     "c5wT", 512, 2048, "c5b", pool=True,
                                     groups=2, sign_tag="s_c5")
                out5s = l5["signs"]

                # local pooled sign (per-batch max over own points)
                yp5 = spool.tile([128, 16], f32, tag="yp5", name="yp5")
                nc.vector.tensor_tensor(yp5[:], l5["maxs_local"][:],
                                        l5["bias_sb"][:], op=ALU.add)
                zp5 = spool.tile([128, 16], f32, tag="zp5", name="zp5")
                nc.vector.tensor_tensor(zp5[:], yp5[:], l5["mean"][:],
                                        op=ALU.subtract)
                pool_s = spool.tile([128, 16], f8, tag="pool_s", name="pool_s")
                nc.scalar.activation(pool_s[:], zp5[:], AF.Sign)
                self.dbg("pool_s", pool_s[:])

                # s1 contrib GEMV (pool block) + label part
                s1pw = self.load_wT("s1wT_pool", 2048, 256, d=f8)
                ps_g = ppool.tile([128, 2], f32, tag="ps", name="ps")
                for m2 in range(2):
                    for ki, (w, kk) in enumerate(s1pw):
                        nc.tensor.matmul(ps_g[:, m2:m2 + 1],
                                         w[:, m2 * 128:(m2 + 1) * 128],
                                         pool_s[:, ki:ki + 1],
                                         start=(ki == 0), stop=(ki == 15))
                lab_c = self.load_vec("s1_labcontrib", 256)
                contrib = spool.tile([128, 2], f32, tag="contrib", name="contrib")
                nc.vector.tensor_tensor(contrib[:], ps_g[:], lab_c[:],
                                        op=ALU.add)
                self.dbg("contrib", contrib[:])

                # s1 main
                w_s1 = []
                for blk, cin in [("o1", 64), ("o2", 128), ("o3", 128),
                                 ("o4", 512)]:
                    w_s1 += self.load_wT(f"s1wT_{blk}", cin, 256, d=f8)
                w_s1 += self.load_wT("s1wT_o5", 2048, 256, d=f8)
                s1_in = ([(t[0][:], t[1]) for t in out1s]
                         + [(t[0][:], t[1]) for t in out2s]
                         + [(t[0][:], t[1]) for t in out3s]
                         + [(t[0][:], t[1]) for t in out4s]
                         + [(t[0][:], t[1]) for t in out5s])
                l = self.conv_layer("s1", s1_in, None, None, 256, "s1b",
                                    preloaded_w=w_s1, sign_tag="tA",
                                    contrib=[contrib[:, 0:1], contrib[:, 1:2]])
                s1s = l["signs"]

                l = self.conv_layer("s2", [(t[0][:], t[1]) for t in s1s],
                                    "s2wT", 256, 256, "s2b", sign_tag="tB")
                s2s = l["signs"]

                l = self.conv_layer("s3", [(t[0][:], t[1]) for t in s2s],
                                    "s3wT", 256, 128, "s3b", value=True,
                                    gamma_name="bns3g", beta_name="bns3be",
                                    sign_tag="tA")
                s3v = l["values"][0][0]

                # s4: fp conv + bias
                s4w = self.load_wT("s4wT", 128, 50, d=f32)
                s4b = self.load_vec("s4b", 50)
                ps4 = ppool.tile([50, npts], f32, tag="ps", name="ps")
                for n in range(ntiles):
                    sl = slice(n * nsub, (n + 1) * nsub)
                    nc.tensor.matmul(ps4[:, sl], s4w[0][0][:], s3v[:, sl],
                                     start=True, stop=True)
                o_sb = spool.tile([50, npts], f32, tag="pcb", name="o_sb")
                nc.scalar.activation(o_sb[:], ps4[:], AF.Identity,
                                     bias=s4b[:50, 0:1], scale=1.0)
                nc.sync.dma_start(out_d.ap(), o_sb[:])

        nc.compile()
        return nc


# ===========================================================================
# host side
# ===========================================================================

def _pad_vec(v, c):
    nch = (c + 127) // 128
    out = np.zeros(nch * 128, np.float32)
    out[:c] = np.asarray(v, np.float32)
    return out


def prep_inputs(point_cloud, label, params, npts=NPTS):
    """Build the 8 per-core in_maps (weights host-preprocessed)."""
    p = params
    sg = lambda a: np.sign(np.asarray(a, np.float32)).astype(np.float32)

    def wT(a, d):
        return np.ascontiguousarray(np.asarray(a, np.float32).T).astype(d)

    def swT(a, d):
        return np.ascontiguousarray(sg(a).T).astype(d)

    bf = _np(bf16)
    f8n = _np(f8)

    com = {}
    # ---- STN1
    s = p['stn']
    com.update({
        "stn_c1wT": swT(s['c1']['w'], f8n), "stn_c1b": _pad_vec(s['c1']['b'], 64),
        "stn_c2wT": swT(s['c2']['w'], f8n), "stn_c2b": _pad_vec(s['c2']['b'], 128),
        "stn_c3wT": swT(s['c3']['w'], f8n), "stn_c3b": _pad_vec(s['c3']['b'], 1024),
        "stn_f1wT": swT(s['f1']['w'], bf), "stn_f1b": _pad_vec(s['f1']['b'], 512),
        "stn_f2wT": swT(s['f2']['w'], bf), "stn_f2b": _pad_vec(s['f2']['b'], 256),
        "stn_f3wT": swT(s['f3']['w'], bf), "stn_f3b": _pad_vec(s['f3']['b'], 9),
        "stn_eye": _pad_vec(np.eye(3, dtype=np.float32).reshape(9), 9),
    })
    # ---- FSTN
    s = p['fstn']
    com.update({
        "fstn_c1wT": swT(s['c1']['w'], f8n), "fstn_c1b": _pad_vec(s['c1']['b'], 64),
        "fstn_c2wT": swT(s['c2']['w'], f8n), "fstn_c2b": _pad_vec(s['c2']['b'], 128),
        "fstn_c3wT": swT(s['c3']['w'], f8n), "fstn_c3b": _pad_vec(s['c3']['b'], 1024),
        "fstn_f1wT": swT(s['f1']['w'], bf), "fstn_f1b": _pad_vec(s['f1']['b'], 512),
        "fstn_f2wT": swT(s['f2']['w'], bf), "fstn_f2b": _pad_vec(s['f2']['b'], 256),
        "fstn_f3wT": swT(s['f3']['w'], f8n),
        "fstn_f3b128": np.asarray(s['f3']['b'], np.float32).reshape(128, 128),
        "fstn_eye128": np.eye(128, dtype=np.float32),
    })
    # ---- main convs
    com.update({
        "c1wT": wT(p['c1']['w'], np.float32), "c1b": _pad_vec(p['c1']['b'], 64),
        "c2wT": swT(p['c2']['w'], f8n), "c2b": _pad_vec(p['c2']['b'], 128),
        "c3wT": swT(p['c3']['w'], f8n), "c3b": _pad_vec(p['c3']['b'], 128),
        "c4wT": swT(p['c4']['w'], f8n), "c4b": _pad_vec(p['c4']['b'], 512),
        "c5wT": swT(p['c5']['w'], f8n), "c5b": _pad_vec(p['c5']['b'], 2048),
        "bn3g": _pad_vec(p['bn3']['g'], 128), "bn3be": _pad_vec(p['bn3']['be'], 128),
        "bns3g": _pad_vec(p['bns3']['g'], 128), "bns3be": _pad_vec(p['bns3']['be'], 128),
    })
    # ---- seg head: s1 split into blocks
    s1w = sg(p['s1']['w'])                       # (256, 4944)
    com.update({
        "s1wT_pool": np.ascontiguousarray(s1w[:, :2048].T).astype(f8n),
        "s1wT_o1": np.ascontiguousarray(s1w[:, 2064:2128].T).astype(f8n),
        "s1wT_o2": np.ascontiguousarray(s1w[:, 2128:2256].T).astype(f8n),
        "s1wT_o3": np.ascontiguousarray(s1w[:, 2256:2384].T).astype(f8n),
        "s1wT_o4": np.ascontiguousarray(s1w[:, 2384:2896].T).astype(f8n),
        "s1wT_o5": np.ascontiguousarray(s1w[:, 2896:4944].T).astype(f8n),
        "s1b": _pad_vec(p['s1']['b'], 256),
        "s2wT": swT(p['s2']['w'], f8n), "s2b": _pad_vec(p['s2']['b'], 256),
        "s3wT": swT(p['s3']['w'], f8n), "s3b": _pad_vec(p['s3']['b'], 128),
        "s4wT": wT(p['s4']['w'], np.float32), "s4b": _pad_vec(p['s4']['b'], 50),
    })

    # sanity: the kernel relies on beta==0 for sign layers and gamma>0
    for bn in [p['stn'][k] for k in ('bn1', 'bn2', 'bn3', 'bn4', 'bn5')] + \
              [p['fstn'][k] for k in ('bn1', 'bn2', 'bn3', 'bn4', 'bn5')] + \
              [p[k] for k in ('bn1', 'bn2', 'bn4', 'bn5', 'bns1', 'bns2')]:
        assert np.all(np.asarray(bn['be']) == 0.0), "kernel assumes beta==0"
        assert np.all(np.asarray(bn['g']) > 0.0), "kernel assumes gamma>0"
    assert np.all(np.asarray(p['bn3']['g']) > 0.0)
    assert np.all(np.asarray(p['bns3']['g']) > 0.0)

    Wl = s1w[:, 2048:2064]                       # (256, 16) label block
    lab_s = np.sign(np.asarray(label, np.float32)[:, 0, :])   # (8, 16)
    lab_contrib = lab_s @ Wl.T                   # (8, 256) ints

    x = np.asarray(point_cloud, np.float32)
    in_maps = []
    for i in range(N_CORES):
        m = dict(com)
        m["x"] = np.ascontiguousarray(x[i, :, :npts])
        m["s1_labcontrib"] = _pad_vec(lab_contrib[i], 256)
        in_maps.append(m)
    return in_maps


_CACHE = {}


def get_prog(npts=NPTS, debug=False):
    key = (npts, debug)
    if key not in _CACHE:
        prog = Prog(npts, debug=debug)
        nc = prog.build()
        _CACHE[key] = (prog, nc)
    return _CACHE[key]


def kernel(point_cloud, label, params):
    prog, nc = get_prog(NPTS)
    in_maps = prep_inputs(point_cloud, label, params)
    # validate shapes/dtypes against declared inputs
    for name, (shape, d) in prog.inputs.items():
        a = in_maps[0][name]
        assert tuple(a.shape) == shape, f"{name}: {a.shape} vs {shape}"
        in_maps = [{**m, name: np.ascontiguousarray(m[name]).astype(d, copy=False)}
                   for m in in_maps]
    res = bass_utils.run_bass_kernel_spmd(nc, in_maps,
                                          core_ids=list(range(N_CORES)))
    out = np.stack([res.results[i]["out"] for i in range(N_CORES)], axis=0)
    return out.astype(np.float32)
